# revision 18
# baseline (speedup 1.0000x reference)
"""DualMemorySystem Trainium2 kernel — 8-core SPMD (batch x 4 row-bands).

Per core: one (b, 32-row out band). Convolution form of unfold/attention/fold:
  sim = conv(x, mem)      -> fp16 matmuls, bg (M=64) and tg (M=8) share one
                             PSUM tile via PE column tiling (tg at col 64)
  E   = exp(sim)          -> one ACT op for both streams, bf16 out
  den = ones-matmul       -> bf16 RHS, 1 cyc/row
  att = E * rcp(den)      -> DVE rcp + DVE (bg) / GpSimd (tg) multiplies
  R   = conv_x(att, mem)  -> fp16 matmuls over col-shifted att replicas
  out = fold_y(R)         -> log-tree of shifted adds; partition moves via
                             SBUF->SBUF DMA on the sync queue
  fusion: per-branch fdiv multiply + pooled partials; host reduces pools
          across the 4 band-cores; phase B applies the tiny MLP + combine.

Phase split (A: branches, B: fusion) because the on-device AllReduce pays
~75us of cross-core launch skew under PJRT.

Hardware facts baked in (probed/traced): fp32r matmul = 2 cyc/row on HW
(fp16/bf16 = 1); matmul rate is set by the MOVING (rhs) dtype; PE runs
matmuls back-to-back at ~0.43ns/row when deps are ready; DVE ops need
32-aligned partition bases; engines cannot remap partitions (only DMA/PE
move data across partitions); DMA issued on an engine queue serializes
with that engine's compute.
"""
import numpy as np
from contextlib import ExitStack

import ml_dtypes

import concourse.bass as bass
import concourse.bacc as bacc
import concourse.tile as tile
from concourse import mybir
from concourse.bass_utils import run_bass_kernel_spmd

F32 = mybir.dt.float32
F16 = mybir.dt.float16
BF16 = mybir.dt.bfloat16

B, C, H, W = 2, 16, 128, 128
PS = (3, 5, 7)
PADS = (1, 2, 3)
NBG, NTG = 64, 8
NCORES = 8
NBANDS = 4
BH = H // NBANDS            # 32 out rows per core
RX = 38                     # x replica rows per core
CX = 134                    # x cols with halo (128 + 6)
RA = 38                     # max att rows (32 + 2*padmax)
RAL = 40                    # R sbuf rows (fold-tree halo)
RAS = [BH + 2 * p for p in PADS]   # att rows per branch: 34, 36, 38

_CACHE = {}

EXP = mybir.ActivationFunctionType.Exp
RELU = mybir.ActivationFunctionType.Relu
COPY = mybir.ActivationFunctionType.Copy


def _build_A():
    nc = bacc.Bacc("TRN2", target_bir_lowering=False, debug=False,
                   num_devices=NCORES)

    d_x8bg = nc.dram_tensor("x8bg", [128, RX, CX], F16, kind="ExternalInput")
    d_x8tg = nc.dram_tensor("x8tg", [128, RX, CX], F16, kind="ExternalInput")
    d_hug = nc.dram_tensor("hug", [3, RA, W], BF16, kind="ExternalInput")
    d_rdiv0 = nc.dram_tensor("rdiv0", [16, 3, BH, W], F16,
                             kind="ExternalInput")
    d_ones = nc.dram_tensor("oneslhs", [73, 72], BF16, kind="ExternalInput")
    d_w1 = {}
    for s, M in (("bg", NBG), ("tg", NTG)):
        for n, p in enumerate(PS):
            d_w1[(s, n)] = nc.dram_tensor(
                f"w1{s}{n}", [16 * p, p, M], F16, kind="ExternalInput")
    d_w2bg = nc.dram_tensor("w2bg", [128, 9, 128], F16, kind="ExternalInput")
    d_w2tg = nc.dram_tensor("w2tg", [64, 3, 128], F16, kind="ExternalInput")
    d_fdiv = nc.dram_tensor("fdiv_out", [96, BH, W], F16,
                            kind="ExternalOutput")
    d_pool = nc.dram_tensor("pool_out", [32], F32, kind="ExternalOutput")

    w2bg_base = [0, 2, 5]

    with tile.TileContext(nc) as tc, ExitStack() as ctx:
        P = ctx.enter_context(tc.tile_pool(name="persist", bufs=1))
        pE = ctx.enter_context(tc.tile_pool(name="epool", bufs=2))
        pRcp = ctx.enter_context(tc.tile_pool(name="rcp", bufs=2))
        pSbg = ctx.enter_context(tc.tile_pool(name="sbg", bufs=2))
        pStg = ctx.enter_context(tc.tile_pool(name="stg", bufs=2))
        pR = ctx.enter_context(tc.tile_pool(name="rsb", bufs=3))
        pS72 = ctx.enter_context(tc.tile_pool(name="s72", bufs=1))
        pAB = ctx.enter_context(tc.tile_pool(name="foldtmp", bufs=1))
        psA = ctx.enter_context(
            tc.tile_pool(name="psA", bufs=2, space=bass.MemorySpace.PSUM))
        psMix = ctx.enter_context(
            tc.tile_pool(name="pmix", bufs=2, space=bass.MemorySpace.PSUM))

        # ---------------- persistent loads ----------------
        x8 = {}
        for s, d, q in (("bg", d_x8bg, nc.sync), ("tg", d_x8tg, nc.gpsimd)):
            t = P.tile([128, RX, CX], F16, tag=f"x8{s}")
            for r0 in range(0, RX, 8):
                r1 = min(r0 + 8, RX)
                q.dma_start(t[:, r0:r1, :], d[:, r0:r1, :])
            x8[s] = t
        w1 = {}
        for s, M in (("bg", NBG), ("tg", NTG)):
            for n, p in enumerate(PS):
                t = P.tile([16 * p, p, M], F16, tag=f"w1{s}{n}")
                nc.scalar.dma_start(t[:], d_w1[(s, n)][:])
                w1[(s, n)] = t
        w2bg = P.tile([128, 9, 128], F16, tag="w2bg")
        nc.scalar.dma_start(w2bg[:], d_w2bg[:])
        w2tg = P.tile([64, 3, 128], F16, tag="w2tg")
        nc.scalar.dma_start(w2tg[:], d_w2tg[:])
        ones_l = P.tile([73, 72], BF16, tag="ones")
        nc.scalar.dma_start(ones_l[:], d_ones[:])
        rdiv0 = P.tile([16, 3, BH, W], F16, tag="rdiv0")
        nc.scalar.dma_start(rdiv0[:], d_rdiv0[:])
        pacc6 = P.tile([16, 8], F32, tag="pacc6")

        def make_branch(n):
            p = PS[n]
            pad = PADS[n]
            Ra = RAS[n]
            rxo = 6 - 2 * pad
            nch = (Ra + 7) // 8
            st = {}

            def c1datt():
                E = pE.tile([73, RA, W], BF16, tag="E")
                nc.sync.dma_start(E[72:73, 0:RA, :], d_hug[n:n + 1, :, :])
                Sbg = pSbg.tile([128, RA, 136], F16, tag="Sbg")
                Stg = pStg.tile([72, RA, 144], F16, tag="Stg")
                S72 = pS72.tile([72, RA, W], F16, tag="S72")
                st["S72"] = S72
                nc.gpsimd.memset(Sbg[:, :, 0:4], 0.0)
                nc.gpsimd.memset(Sbg[:, :, 131:136], 0.0)
                nc.gpsimd.memset(Stg[:, :, 0:15], 0.0)
                nc.gpsimd.memset(Stg[:, :, 136:144], 0.0)
                st["E"], st["Sbg"], st["Stg"] = E, Sbg, Stg

                def conv1_chunk(ch):
                    r0 = 8 * ch
                    rr = min(8, Ra - r0)
                    ps = psA.tile([72, 8, W], F32, tag="c1")
                    for h in range(0, rr, 4):
                        hh = min(4, rr - h)
                        for j in range(p):
                            nc.tensor.matmul(
                                ps[0:64, h:h + hh, :],
                                w1[("bg", n)][:, j, :],
                                x8["bg"][0:16 * p,
                                         r0 + h + rxo:r0 + h + rxo + hh,
                                         j + 3 - pad:j + 3 - pad + W],
                                start=(j == 0), stop=(j == p - 1))
                        for j in range(p):
                            nc.tensor.matmul(
                                ps[64:72, h:h + hh, :],
                                w1[("tg", n)][:, j, :],
                                x8["tg"][0:16 * p,
                                         r0 + h + rxo:r0 + h + rxo + hh,
                                         j + 3 - pad:j + 3 - pad + W],
                                start=(j == 0), stop=(j == p - 1))
                    nc.scalar.activation(E[0:72, r0:r0 + rr, :],
                                         ps[0:72, 0:rr, :], EXP)

                def datt_chunk(ch):
                    r0 = 8 * ch
                    rr = min(8, Ra - r0)
                    sb = psMix.tile([128, 8, W], F32, tag="mix")
                    for h in range(0, rr, 4):
                        hh = min(4, rr - h)
                        nc.tensor.matmul(
                            sb[0:72, h:h + hh, :],
                            ones_l[:, 0:72],
                            E[:, r0 + h:r0 + h + hh, :],
                            start=True, stop=True)
                    rcp = pRcp.tile([72, 8, W], F32, tag="rcp")
                    nc.vector.reciprocal_approx_fast(rcp[0:72, 0:rr, :],
                                                     sb[0:72, 0:rr, :])
                    nc.vector.tensor_mul(S72[0:72, r0:r0 + rr, :],
                                         E[0:72, r0:r0 + rr, :],
                                         rcp[0:72, 0:rr, :])
                    nc.sync.dma_start(Sbg[0:64, r0:r0 + rr, 3:131],
                                      S72[0:64, r0:r0 + rr, :])
                    nc.sync.dma_start(Sbg[64:128, r0:r0 + rr, 4:132],
                                      S72[0:64, r0:r0 + rr, :])

                for ch in range(nch):
                    conv1_chunk(ch)
                    if ch >= 1:
                        datt_chunk(ch - 1)
                datt_chunk(nch - 1)
                for g in range(p):
                    nc.gpsimd.dma_start(Stg[8 * g:8 * g + 8, 0:Ra,
                                            8 + g:136 + g],
                                        S72[64:72, 0:Ra, :])

            def conv2():
                Sbg, Stg = st["Sbg"], st["Stg"]
                Rtg = pR.tile([128, RAL, W], F16, tag="R")
                Rbg = pR.tile([128, RAL, W], F16, tag="R")
                nc.gpsimd.memset(Rtg[:, Ra:RAL, :], 0.0)
                nc.gpsimd.memset(Rbg[:, Ra:RAL, :], 0.0)
                nchk = (p + 1) // 2
                for tl in range(nch):
                    r0 = 8 * tl
                    rr = min(8, Ra - r0)
                    rp = psMix.tile([128, 8, W], F32, tag="mix")
                    for h in range(0, rr, 4):
                        hh = min(4, rr - h)
                        nc.tensor.matmul(
                            rp[0:128, h:h + hh, :],
                            w2tg[0:8 * p, n, :],
                            Stg[0:8 * p, r0 + h:r0 + h + hh,
                                8 + pad:8 + pad + W],
                            start=True, stop=True)
                    nc.scalar.activation(Rtg[:, r0:r0 + rr, :], rp[:, 0:rr, :],
                                         COPY)
                for tl in range(nch):
                    r0 = 8 * tl
                    rr = min(8, Ra - r0)
                    rp = psMix.tile([128, 8, W], F32, tag="mix")
                    for h in range(0, rr, 4):
                        hh = min(4, rr - h)
                        for ci in range(nchk):
                            jj = 2 * ci
                            nc.tensor.matmul(
                                rp[:, h:h + hh, :],
                                w2bg[:, w2bg_base[n] + ci, :],
                                Sbg[:, r0 + h:r0 + h + hh,
                                    3 + pad - jj:3 + pad - jj + W],
                                start=(ci == 0), stop=(ci == nchk - 1))
                    nc.scalar.activation(Rbg[:, r0:r0 + rr, :], rp[:, 0:rr, :],
                                         COPY)
                st["Rbg"], st["Rtg"] = Rbg, Rtg

            def fold(Rsb, dst, q, eng):
                if p == 3:
                    Rs1 = pAB.tile([64, 35, W], F16, tag="S1")
                    q.dma_start(Rs1[0:32, 0:33, :], Rsb[32:64, 2:35, :])
                    A = pAB.tile([64, 35, W], F16, tag="A")
                    eng.tensor_add(A[0:32, 0:33, :], Rsb[0:32, 0:33, :],
                                   Rs1[0:32, 0:33, :])
                    As1 = pAB.tile([16, 33, W], F16, tag="B")
                    q.dma_start(As1[0:16, 0:BH, :], A[16:32, 1:BH + 1, :])
                    eng.tensor_add(dst, A[0:16, 0:BH, :],
                                   As1[0:16, 0:BH, :])
                else:
                    Rs1 = pAB.tile([64, 35, W], F16, tag="S1")
                    q.dma_start(Rs1[:, 0:35, :], Rsb[64:128, 4:39, :])
                    A = pAB.tile([64, 35, W], F16, tag="A")
                    eng.tensor_add(A[:, 0:35, :], Rsb[0:64, 0:35, :],
                                   Rs1[:, 0:35, :])
                    As1 = pAB.tile([32, 33, W], F16, tag="S1")
                    q.dma_start(As1[0:32, 0:33, :], A[32:64, 2:35, :])
                    Bt = pAB.tile([32, 33, W], F16, tag="B")
                    eng.tensor_add(Bt[:, 0:33, :], A[0:32, 0:33, :],
                                   As1[0:32, 0:33, :])
                    Bs1 = pAB.tile([16, 33, W], F16, tag="S1")
                    q.dma_start(Bs1[0:16, 0:BH, :], Bt[16:32, 1:BH + 1, :])
                    eng.tensor_add(dst, Bt[0:16, 0:BH, :],
                                   Bs1[0:16, 0:BH, :])

            def foldstt():
                ftT = pAB.tile([16, BH, W], F16, tag="ftT")
                fold(st["Rtg"], ftT[0:16, :, :], nc.gpsimd, nc.gpsimd)
                fvT = pAB.tile([16, BH, W], F16, tag="fvT")
                nc.vector.scalar_tensor_tensor(
                    fvT[:], ftT[:], 0.0, rdiv0[:, n, :, :],
                    op0=mybir.AluOpType.bypass, op1=mybir.AluOpType.mult,
                    accum_out=pacc6[:, 2 * n + 1:2 * n + 2])
                nc.gpsimd.dma_start(d_fdiv[32 * n + 16:32 * n + 32, :, :],
                                    fvT[:])
                ftB = pAB.tile([16, BH, W], F16, tag="ftT")
                fold(st["Rbg"], ftB[0:16, :, :], nc.sync, nc.vector)
                fvB = pAB.tile([16, BH, W], F16, tag="fvT")
                nc.vector.scalar_tensor_tensor(
                    fvB[:], ftB[:], 0.0, rdiv0[:, n, :, :],
                    op0=mybir.AluOpType.bypass, op1=mybir.AluOpType.mult,
                    accum_out=pacc6[:, 2 * n:2 * n + 1])
                nc.sync.dma_start(d_fdiv[32 * n:32 * n + 16, :, :], fvB[:])

            st["c1datt"], st["conv2"], st["foldstt"] = c1datt, conv2, foldstt
            return st

        br = {n: make_branch(n) for n in (2, 1, 0)}
        br[2]["c1datt"]()
        br[2]["conv2"]()
        br[1]["c1datt"]()
        br[2]["foldstt"]()
        br[1]["conv2"]()
        br[0]["c1datt"]()
        br[1]["foldstt"]()
        br[0]["conv2"]()
        br[0]["foldstt"]()

        # ---------------- pool finalize ----------------
        tb = P.tile([16, 2], F32, tag="tb")
        pbg = P.tile([16, 1], F32, tag="pbg")
        ptg = P.tile([16, 1], F32, tag="ptg")
        nc.vector.tensor_add(tb[:, 0:1], pacc6[:, 0:1], pacc6[:, 2:3])
        nc.vector.tensor_add(pbg[:], tb[:, 0:1], pacc6[:, 4:5])
        nc.vector.tensor_add(tb[:, 1:2], pacc6[:, 1:2], pacc6[:, 3:4])
        nc.vector.tensor_add(ptg[:], tb[:, 1:2], pacc6[:, 5:6])
        nc.sync.dma_start(d_pool[0:16], pbg[:, 0])
        nc.sync.dma_start(d_pool[16:32], ptg[:, 0])

    nc.compile()
    return nc


def _build_B():
    nc = bacc.Bacc("TRN2", target_bir_lowering=False, debug=False,
                   num_devices=NCORES)

    d_fdiv = nc.dram_tensor("fdiv_in", [96, BH, W], F16, kind="ExternalInput")
    d_poolg = nc.dram_tensor("poolg", [80], F32, kind="ExternalInput")
    d_mw1 = nc.dram_tensor("mlpw1t", [80, 8], F32, kind="ExternalInput")
    d_mb1 = nc.dram_tensor("mlpb1", [8, 1], F32, kind="ExternalInput")
    d_mw2 = nc.dram_tensor("mlpw2t", [8, 96], F32, kind="ExternalInput")
    d_mb2 = nc.dram_tensor("mlpb2", [96, 1], F32, kind="ExternalInput")
    d_ones32 = nc.dram_tensor("ones32", [96, 32], F32, kind="ExternalInput")
    d_perm96 = nc.dram_tensor("perm96", [32, 96], F32, kind="ExternalInput")
    d_pat16 = nc.dram_tensor("pat16", [96, 32], F16, kind="ExternalInput")
    d_obg = nc.dram_tensor("out_bg", [C, BH, W], F16, kind="ExternalOutput")
    d_otg = nc.dram_tensor("out_tg", [C, BH, W], F16, kind="ExternalOutput")

    with tile.TileContext(nc) as tc, ExitStack() as ctx:
        P = ctx.enter_context(tc.tile_pool(name="persist", bufs=1))
        psB = ctx.enter_context(
            tc.tile_pool(name="psB", bufs=2, space=bass.MemorySpace.PSUM))
        psM = ctx.enter_context(
            tc.tile_pool(name="psM", bufs=2, space=bass.MemorySpace.PSUM))

        fdiv = P.tile([96, BH, W], F16, tag="fdiv")
        qs = [nc.sync, nc.gpsimd, nc.scalar, nc.sync]
        for q_i, q in enumerate(qs):
            r0 = 8 * q_i
            q.dma_start(fdiv[:, r0:r0 + 8, :], d_fdiv[:, r0:r0 + 8, :])
        poolg = P.tile([80, 1], F32, tag="poolg")
        nc.sync.dma_start(poolg[:, 0], d_poolg[:])
        mw1 = P.tile([80, 8], F32, tag="mw1")
        nc.scalar.dma_start(mw1[:], d_mw1[:])
        mb1 = P.tile([8, 1], F32, tag="mb1")
        nc.scalar.dma_start(mb1[:], d_mb1[:])
        mw2 = P.tile([8, 96], F32, tag="mw2")
        nc.scalar.dma_start(mw2[:], d_mw2[:])
        mb2 = P.tile([96, 1], F32, tag="mb2")
        nc.scalar.dma_start(mb2[:], d_mb2[:])
        ones32 = P.tile([96, 32], F32, tag="ones32")
        nc.gpsimd.dma_start(ones32[:], d_ones32[:])
        perm96 = P.tile([32, 96], F32, tag="perm96")
        nc.gpsimd.dma_start(perm96[:], d_perm96[:])
        pat16 = P.tile([96, 32], F16, tag="pat16")
        nc.gpsimd.dma_start(pat16[:], d_pat16[:])

        # merged two-stream MLP (block-diagonal weights), all N=1
        pm1 = psM.tile([96, 4], F32, tag="m")
        nc.tensor.matmul(pm1[0:8, 0:1], mw1[:, 0:8], poolg[:, 0:1],
                         start=True, stop=True)
        hdn = P.tile([8, 1], F32, tag="hdn")
        nc.scalar.activation(hdn[:], pm1[0:8, 0:1], RELU, bias=mb1[:])
        pm2 = psM.tile([96, 4], F32, tag="m")
        nc.tensor.matmul(pm2[0:96, 0:1], mw2[:, 0:96], hdn[:, 0:1],
                         start=True, stop=True)
        elog = P.tile([96, 1], F32, tag="elog")
        nc.scalar.activation(elog[:], pm2[0:96, 0:1], EXP, bias=mb2[:])
        pm3 = psM.tile([96, 4], F32, tag="m")
        nc.tensor.matmul(pm3[0:32, 0:1], ones32[:, 0:32], elog[:, 0:1],
                         start=True, stop=True)
        s3r = P.tile([32, 1], F32, tag="s3r")
        nc.vector.reciprocal(s3r[:], pm3[0:32, 0:1])
        pm4 = psM.tile([96, 4], F32, tag="m")
        nc.tensor.matmul(pm4[0:96, 0:1], perm96[:, 0:96], s3r[:, 0:1],
                         start=True, stop=True)
        wtl = P.tile([96, 1], F32, tag="wtl")
        nc.vector.tensor_mul(wtl[:], elog[:], pm4[0:96, 0:1])
        lhs96 = P.tile([96, 32], F16, tag="lhs96")
        nc.vector.tensor_scalar_mul(lhs96[:], pat16[:], wtl[:])

        # weighted combine on the PE: out[(si,c)] = sum_k wt[k]*fdiv[k]
        obuf = P.tile([32, BH, W], F16, tag="obuf")
        for q_i in range(4):
            r0 = 8 * q_i
            pq = psB.tile([32, 8, W], F32, tag="cmb")
            for h in (0, 4):
                nc.tensor.matmul(pq[0:32, h:h + 4, :], lhs96[:, 0:32],
                                 fdiv[:, r0 + h:r0 + h + 4, :],
                                 start=True, stop=True)
            nc.scalar.activation(obuf[:, r0:r0 + 8, :], pq[0:32, 0:8, :],
                                 COPY)
        nc.sync.dma_start(d_obg[:], obuf[0:16, :, :])
        nc.gpsimd.dma_start(d_otg[:], obuf[16:32, :, :])

    nc.compile()
    return nc


# ======================= host-side prep =======================

def _prep_core(inputs, b, k):
    y0 = BH * k
    m = {}
    for s in ("bg", "tg"):
        x = np.asarray(inputs[s])[b]            # [C, H, W]
        x8 = np.zeros((8, C, RX, CX), np.float16)
        for g in range(8):
            lo = y0 - 6 + g
            hi = lo + RX
            slo, shi = max(lo, 0), min(hi, H)
            if slo < shi:
                x8[g, :, slo - lo:shi - lo, 3:131] = x[:, slo:shi, :]
        m[f"x8{s}"] = x8.reshape(128, RX, CX)

    hug = np.zeros((3, RA, W), np.float32)
    for n, pad in enumerate(PADS):
        for r in range(RA):
            y = y0 - pad + r
            if not (0 <= y < H):
                hug[n, r, :] = 1e30
    m["hug"] = hug.astype(ml_dtypes.bfloat16)

    rdiv0 = np.zeros((16, 3, BH, W), np.float32)
    for n, pad in enumerate(PADS):
        yy = np.arange(H)
        rc = np.minimum(yy, pad) + np.minimum(H - 1 - yy, pad) + 1.0
        cc = np.minimum(yy[:W], pad) + np.minimum(W - 1 - yy[:W], pad) + 1.0
        div = np.outer(rc[y0:y0 + BH], cc) + 1e-8
        rdiv0[:, n] = (1.0 / div)[None, :, :]
    m["rdiv0"] = rdiv0.astype(np.float16)

    ones = np.zeros((73, 72), np.float32)
    ones[0:64, 0:64] = 1.0
    ones[64:72, 64:72] = 1.0
    ones[72, :] = 1.0
    m["oneslhs"] = ones.astype(ml_dtypes.bfloat16)

    for s, M, nmem in (("bg", NBG, "bg_mem"), ("tg", NTG, "tg_mem")):
        for n, p in enumerate(PS):
            mem = np.asarray(inputs[f"{nmem}{n}"])          # [M, C*p*p]
            temp = float(np.asarray(inputs[f"{s}_temp{n}"])[0])
            D = C * p * p
            arr = mem.reshape(M, C, p, p)
            w1 = arr.transpose(2, 1, 3, 0).reshape(p * C, p, M).copy()
            m[f"w1{s}{n}"] = (w1 * (temp / np.sqrt(D))).astype(np.float16)

    # fold consumes group q at row shift +q where q = 2*pad - i
    w2bg = np.zeros((2, NBG, 9, 8, 16), np.float32)
    base = [0, 2, 5]
    for n, p in enumerate(PS):
        pad = PADS[n]
        arr = np.asarray(inputs[f"bg_mem{n}"]).reshape(NBG, C, p, p)
        for ci in range((p + 1) // 2):
            for g in range(2):
                j = 2 * ci + g
                if j < p:
                    for i in range(p):
                        w2bg[g, :, base[n] + ci, 2 * pad - i, :] = \
                            arr[:, :, i, j]
    m["w2bg"] = w2bg.reshape(128, 9, 128).astype(np.float16)

    w2tg = np.zeros((8, NTG, 3, 8, 16), np.float32)
    for n, p in enumerate(PS):
        pad = PADS[n]
        arr = np.asarray(inputs[f"tg_mem{n}"]).reshape(NTG, C, p, p)
        for g in range(p):
            for i in range(p):
                w2tg[g, :, n, 2 * pad - i, :] = arr[:, :, i, g]
    m["w2tg"] = w2tg.reshape(64, 3, 128).astype(np.float16)
    return m


def _prep_B(inputs):
    """Per-batch-independent phase-B tensors (row layout: 32n+16si+c)."""
    m = {}
    mw1 = np.zeros((80, 8), np.float32)
    mb1 = np.zeros((8, 1), np.float32)
    mw2 = np.zeros((8, 96), np.float32)
    mb2 = np.zeros((96, 1), np.float32)
    for si, s in enumerate(("bg", "tg")):
        mw1[64 * si:64 * si + 16, 4 * si:4 * si + 4] = (
            np.asarray(inputs[f"{s}_fc1_w"]).T / (H * W))
        mb1[4 * si:4 * si + 4, 0] = np.asarray(inputs[f"{s}_fc1_b"])
        w2 = np.asarray(inputs[f"{s}_fc2_w"])          # [48, 4], row 16n+c
        b2 = np.asarray(inputs[f"{s}_fc2_b"])
        for n in range(3):
            for c in range(C):
                row = 32 * n + 16 * si + c
                mw2[4 * si:4 * si + 4, row] = w2[16 * n + c, :]
                mb2[row, 0] = b2[16 * n + c]
    ones32 = np.zeros((96, 32), np.float32)
    for n in range(3):
        for si in range(2):
            for c in range(C):
                ones32[32 * n + 16 * si + c, 16 * si + c] = 1.0
    m["mlpw1t"], m["mlpb1"] = mw1, mb1
    m["mlpw2t"], m["mlpb2"] = mw2, mb2
    m["ones32"] = ones32
    m["perm96"] = ones32.T.copy()
    m["pat16"] = ones32.astype(np.float16)
    return m


TRACE = False
TRACE_DIR = None


def kernel(**inputs):
    if "ncA" not in _CACHE:
        _CACHE["ncA"] = _build_A()
        _CACHE["ncB"] = _build_B()

    in_maps = []
    for core in range(NCORES):
        b, k = divmod(core, NBANDS)
        in_maps.append(_prep_core(inputs, b, k))

    kwA = {}
    kwB = {}
    if TRACE:
        kwA = dict(trace=True, tmpdir=(TRACE_DIR + "/A") if TRACE_DIR else None)
        kwB = dict(trace=True, tmpdir=(TRACE_DIR + "/B") if TRACE_DIR else None)

    resA = run_bass_kernel_spmd(_CACHE["ncA"], in_maps, list(range(NCORES)),
                                **kwA)

    # host glue: reduce pooled partials within each batch's 4-band group
    poolsum = {}
    for b in range(B):
        ps = np.sum([resA.results[b * NBANDS + k]["pool_out"]
                     for k in range(NBANDS)], axis=0).astype(np.float32)
        pg = np.zeros(80, np.float32)
        pg[0:16] = ps[0:16]
        pg[64:80] = ps[16:32]
        poolsum[b] = pg

    mB = _prep_B(inputs)
    in_mapsB = []
    for core in range(NCORES):
        b, k = divmod(core, NBANDS)
        d = dict(mB)
        d["fdiv_in"] = resA.results[core]["fdiv_out"]
        d["poolg"] = poolsum[b]
        in_mapsB.append(d)
    resB = run_bass_kernel_spmd(_CACHE["ncB"], in_mapsB, list(range(NCORES)),
                                **kwB)
    _CACHE["resA"] = resA
    _CACHE["resB"] = resB

    f_bc = np.zeros((B, C, H, W), np.float32)
    f_tg = np.zeros((B, C, H, W), np.float32)
    for core in range(NCORES):
        b, k = divmod(core, NBANDS)
        y0 = BH * k
        f_bc[b, :, y0:y0 + BH, :] = resB.results[core]["out_bg"].astype(np.float32)
        f_tg[b, :, y0:y0 + BH, :] = resB.results[core]["out_tg"].astype(np.float32)
    return (f_bc, f_tg)


# revision 21
# speedup vs baseline: 1.1714x; 1.1714x over previous
"""DualMemorySystem Trainium2 kernel — 8-core SPMD (batch x 4 row-bands).

Per core: one (b, 32-row out band). Convolution form of unfold/attention/fold:
  sim = conv(x, mem)      -> fp16 matmuls, bg (M=64) and tg (M=8) share one
                             PSUM tile via PE column tiling (tg at col 64)
  E   = exp(sim)          -> one ACT op for both streams, bf16 out
  den = ones-matmul       -> bf16 RHS, 1 cyc/row
  att = E * rcp(den)      -> DVE rcp + DVE (bg) / GpSimd (tg) multiplies
  R   = conv_x(att, mem)  -> fp16 matmuls over col-shifted att replicas
  out = fold_y(R)         -> log-tree of shifted adds; partition moves via
                             SBUF->SBUF DMA on the sync queue
  fusion: per-branch fdiv multiply + pooled partials; host reduces pools
          across the 4 band-cores; phase B applies the tiny MLP + combine.

Phase split (A: branches, B: fusion) because the on-device AllReduce pays
~75us of cross-core launch skew under PJRT.

Hardware facts baked in (probed/traced): fp32r matmul = 2 cyc/row on HW
(fp16/bf16 = 1); matmul rate is set by the MOVING (rhs) dtype; PE runs
matmuls back-to-back at ~0.43ns/row when deps are ready; DVE ops need
32-aligned partition bases; engines cannot remap partitions (only DMA/PE
move data across partitions); DMA issued on an engine queue serializes
with that engine's compute.
"""
import numpy as np
from contextlib import ExitStack

import ml_dtypes

import concourse.bass as bass
import concourse.bacc as bacc
import concourse.tile as tile
from concourse import mybir
from concourse.bass_utils import run_bass_kernel_spmd

F32 = mybir.dt.float32
F16 = mybir.dt.float16
BF16 = mybir.dt.bfloat16

B, C, H, W = 2, 16, 128, 128
PS = (3, 5, 7)
PADS = (1, 2, 3)
NBG, NTG = 64, 8
NCORES = 8
NBANDS = 4
BH = H // NBANDS            # 32 out rows per core
RX = 38                     # x replica rows per core
CX = 134                    # x cols with halo (128 + 6)
RA = 38                     # max att rows (32 + 2*padmax)
RAL = 40                    # R sbuf rows (fold-tree halo)
RAS = [BH + 2 * p for p in PADS]   # att rows per branch: 34, 36, 38

_CACHE = {}

EXP = mybir.ActivationFunctionType.Exp
RELU = mybir.ActivationFunctionType.Relu
COPY = mybir.ActivationFunctionType.Copy
RCP = mybir.ActivationFunctionType.Reciprocal


def _build_A():
    nc = bacc.Bacc("TRN2", target_bir_lowering=False, debug=False,
                   num_devices=NCORES)

    d_x8bg = nc.dram_tensor("x8bg", [128, RX, CX], F16, kind="ExternalInput")
    d_x8tg = nc.dram_tensor("x8tg", [128, RX, CX], F16, kind="ExternalInput")
    d_hug = nc.dram_tensor("hug", [3, RA, W], BF16, kind="ExternalInput")
    d_rdiv0 = nc.dram_tensor("rdiv0", [16, 3, BH, W], F16,
                             kind="ExternalInput")
    d_ones = nc.dram_tensor("oneslhs", [73, 72], BF16, kind="ExternalInput")
    d_w1 = {}
    for s, M in (("bg", NBG), ("tg", NTG)):
        for n, p in enumerate(PS):
            d_w1[(s, n)] = nc.dram_tensor(
                f"w1{s}{n}", [16 * p, p, M], F16, kind="ExternalInput")
    d_w2bg = nc.dram_tensor("w2bg", [128, 9, 128], F16, kind="ExternalInput")
    d_w2tg = nc.dram_tensor("w2tg", [64, 3, 128], F16, kind="ExternalInput")
    d_fdiv = nc.dram_tensor("fdiv_out", [96, BH, W], F16,
                            kind="ExternalOutput")
    d_pool = nc.dram_tensor("pool_out", [32], F32, kind="ExternalOutput")

    w2bg_base = [0, 2, 5]

    with tile.TileContext(nc) as tc, ExitStack() as ctx:
        P = ctx.enter_context(tc.tile_pool(name="persist", bufs=1))
        pE = ctx.enter_context(tc.tile_pool(name="epool", bufs=2))
        pRcp = ctx.enter_context(tc.tile_pool(name="rcp", bufs=2))
        pSbg = ctx.enter_context(tc.tile_pool(name="sbg", bufs=2))
        pStg = ctx.enter_context(tc.tile_pool(name="stg", bufs=2))
        pR = ctx.enter_context(tc.tile_pool(name="rsb", bufs=3))
        pS72 = ctx.enter_context(tc.tile_pool(name="s72", bufs=1))
        pAB = ctx.enter_context(tc.tile_pool(name="foldtmp", bufs=1))
        psA = ctx.enter_context(
            tc.tile_pool(name="psA", bufs=2, space=bass.MemorySpace.PSUM))
        psMix = ctx.enter_context(
            tc.tile_pool(name="pmix", bufs=2, space=bass.MemorySpace.PSUM))

        # ---------------- persistent loads ----------------
        x8 = {}
        for s, d, q in (("bg", d_x8bg, nc.sync), ("tg", d_x8tg, nc.gpsimd)):
            t = P.tile([128, RX, CX], F16, tag=f"x8{s}")
            for r0 in range(0, RX, 8):
                r1 = min(r0 + 8, RX)
                q.dma_start(t[:, r0:r1, :], d[:, r0:r1, :])
            x8[s] = t
        w1 = {}
        for s, M in (("bg", NBG), ("tg", NTG)):
            for n, p in enumerate(PS):
                t = P.tile([16 * p, p, M], F16, tag=f"w1{s}{n}")
                nc.scalar.dma_start(t[:], d_w1[(s, n)][:])
                w1[(s, n)] = t
        w2bg = P.tile([128, 9, 128], F16, tag="w2bg")
        nc.scalar.dma_start(w2bg[:], d_w2bg[:])
        w2tg = P.tile([64, 3, 128], F16, tag="w2tg")
        nc.scalar.dma_start(w2tg[:], d_w2tg[:])
        ones_l = P.tile([73, 72], BF16, tag="ones")
        nc.scalar.dma_start(ones_l[:], d_ones[:])
        rdiv0 = P.tile([16, 3, BH, W], F16, tag="rdiv0")
        nc.scalar.dma_start(rdiv0[:], d_rdiv0[:])
        pacc6 = P.tile([16, 8], F32, tag="pacc6")

        def make_branch(n):
            p = PS[n]
            pad = PADS[n]
            Ra = RAS[n]
            rxo = 6 - 2 * pad
            nch = (Ra + 7) // 8
            st = {}

            def c1datt():
                E = pE.tile([73, RA, W], BF16, tag="E")
                nc.sync.dma_start(E[72:73, 0:RA, :], d_hug[n:n + 1, :, :])
                Sbg = pSbg.tile([128, RA, 136], F16, tag="Sbg")
                Stg = pStg.tile([72, RA, 144], F16, tag="Stg")
                S72 = pS72.tile([72, RA, W], F16, tag="S72")
                st["S72"] = S72
                nc.gpsimd.memset(Sbg[:, :, 0:4], 0.0)
                nc.gpsimd.memset(Sbg[:, :, 131:136], 0.0)
                nc.gpsimd.memset(Stg[:, :, 0:15], 0.0)
                nc.gpsimd.memset(Stg[:, :, 136:144], 0.0)
                st["E"], st["Sbg"], st["Stg"] = E, Sbg, Stg

                def conv1_chunk(ch):
                    r0 = 8 * ch
                    rr = min(8, Ra - r0)
                    ps = psA.tile([72, 8, W], F32, tag="c1")
                    for h in range(0, rr, 4):
                        hh = min(4, rr - h)
                        for j in range(p):
                            nc.tensor.matmul(
                                ps[0:64, h:h + hh, :],
                                w1[("bg", n)][:, j, :],
                                x8["bg"][0:16 * p,
                                         r0 + h + rxo:r0 + h + rxo + hh,
                                         j + 3 - pad:j + 3 - pad + W],
                                start=(j == 0), stop=(j == p - 1))
                        for j in range(p):
                            nc.tensor.matmul(
                                ps[64:72, h:h + hh, :],
                                w1[("tg", n)][:, j, :],
                                x8["tg"][0:16 * p,
                                         r0 + h + rxo:r0 + h + rxo + hh,
                                         j + 3 - pad:j + 3 - pad + W],
                                start=(j == 0), stop=(j == p - 1))
                    nc.scalar.activation(E[0:72, r0:r0 + rr, :],
                                         ps[0:72, 0:rr, :], EXP)

                def datt_chunk(ch):
                    r0 = 8 * ch
                    rr = min(8, Ra - r0)
                    sb = psMix.tile([128, 8, W], F32, tag="mix")
                    for h in range(0, rr, 4):
                        hh = min(4, rr - h)
                        nc.tensor.matmul(
                            sb[0:72, h:h + hh, :],
                            ones_l[:, 0:72],
                            E[:, r0 + h:r0 + h + hh, :],
                            start=True, stop=True)
                    rcp = pRcp.tile([72, 8, W], F32, tag="rcp")
                    nc.vector.reciprocal_approx_fast(rcp[0:72, 0:rr, :],
                                                     sb[0:72, 0:rr, :])
                    nc.vector.tensor_mul(S72[0:72, r0:r0 + rr, :],
                                         E[0:72, r0:r0 + rr, :],
                                         rcp[0:72, 0:rr, :])
                    nc.sync.dma_start(Sbg[0:64, r0:r0 + rr, 3:131],
                                      S72[0:64, r0:r0 + rr, :])
                    nc.sync.dma_start(Sbg[64:128, r0:r0 + rr, 4:132],
                                      S72[0:64, r0:r0 + rr, :])

                for ch in range(nch):
                    conv1_chunk(ch)
                    if ch >= 1:
                        datt_chunk(ch - 1)
                datt_chunk(nch - 1)
                for g in range(p):
                    nc.gpsimd.dma_start(Stg[8 * g:8 * g + 8, 0:Ra,
                                            8 + g:136 + g],
                                        S72[64:72, 0:Ra, :])

            def conv2():
                Sbg, Stg = st["Sbg"], st["Stg"]
                Rtg = pR.tile([128, RAL, W], F16, tag="R")
                Rbg = pR.tile([128, RAL, W], F16, tag="R")
                nc.gpsimd.memset(Rtg[:, Ra:RAL, :], 0.0)
                nc.gpsimd.memset(Rbg[:, Ra:RAL, :], 0.0)
                nchk = (p + 1) // 2
                for tl in range(nch):
                    r0 = 8 * tl
                    rr = min(8, Ra - r0)
                    rp = psMix.tile([128, 8, W], F32, tag="mix")
                    for h in range(0, rr, 4):
                        hh = min(4, rr - h)
                        nc.tensor.matmul(
                            rp[0:128, h:h + hh, :],
                            w2tg[0:8 * p, n, :],
                            Stg[0:8 * p, r0 + h:r0 + h + hh,
                                8 + pad:8 + pad + W],
                            start=True, stop=True)
                    nc.scalar.activation(Rtg[:, r0:r0 + rr, :], rp[:, 0:rr, :],
                                         COPY)
                for tl in range(nch):
                    r0 = 8 * tl
                    rr = min(8, Ra - r0)
                    rp = psMix.tile([128, 8, W], F32, tag="mix")
                    for h in range(0, rr, 4):
                        hh = min(4, rr - h)
                        for ci in range(nchk):
                            jj = 2 * ci
                            nc.tensor.matmul(
                                rp[:, h:h + hh, :],
                                w2bg[:, w2bg_base[n] + ci, :],
                                Sbg[:, r0 + h:r0 + h + hh,
                                    3 + pad - jj:3 + pad - jj + W],
                                start=(ci == 0), stop=(ci == nchk - 1))
                    nc.scalar.activation(Rbg[:, r0:r0 + rr, :], rp[:, 0:rr, :],
                                         COPY)
                st["Rbg"], st["Rtg"] = Rbg, Rtg

            def fold(Rsb, dst, q, eng):
                if p == 3:
                    Rs1 = pAB.tile([64, 35, W], F16, tag="S1")
                    q.dma_start(Rs1[0:32, 0:33, :], Rsb[32:64, 2:35, :])
                    A = pAB.tile([64, 35, W], F16, tag="A")
                    eng.tensor_add(A[0:32, 0:33, :], Rsb[0:32, 0:33, :],
                                   Rs1[0:32, 0:33, :])
                    As1 = pAB.tile([16, 33, W], F16, tag="B")
                    q.dma_start(As1[0:16, 0:BH, :], A[16:32, 1:BH + 1, :])
                    eng.tensor_add(dst, A[0:16, 0:BH, :],
                                   As1[0:16, 0:BH, :])
                else:
                    Rs1 = pAB.tile([64, 35, W], F16, tag="S1")
                    q.dma_start(Rs1[:, 0:35, :], Rsb[64:128, 4:39, :])
                    A = pAB.tile([64, 35, W], F16, tag="A")
                    eng.tensor_add(A[:, 0:35, :], Rsb[0:64, 0:35, :],
                                   Rs1[:, 0:35, :])
                    As1 = pAB.tile([32, 33, W], F16, tag="S1")
                    q.dma_start(As1[0:32, 0:33, :], A[32:64, 2:35, :])
                    Bt = pAB.tile([32, 33, W], F16, tag="B")
                    eng.tensor_add(Bt[:, 0:33, :], A[0:32, 0:33, :],
                                   As1[0:32, 0:33, :])
                    Bs1 = pAB.tile([16, 33, W], F16, tag="S1")
                    q.dma_start(Bs1[0:16, 0:BH, :], Bt[16:32, 1:BH + 1, :])
                    eng.tensor_add(dst, Bt[0:16, 0:BH, :],
                                   Bs1[0:16, 0:BH, :])

            def foldstt():
                ftT = pAB.tile([16, BH, W], F16, tag="ftT")
                fold(st["Rtg"], ftT[0:16, :, :], nc.gpsimd, nc.vector)
                fvT = pAB.tile([16, BH, W], F16, tag="fvT")
                nc.vector.scalar_tensor_tensor(
                    fvT[:], ftT[:], 0.0, rdiv0[:, n, :, :],
                    op0=mybir.AluOpType.bypass, op1=mybir.AluOpType.mult,
                    accum_out=pacc6[:, 2 * n + 1:2 * n + 2])
                nc.gpsimd.dma_start(d_fdiv[32 * n + 16:32 * n + 32, :, :],
                                    fvT[:])
                ftB = pAB.tile([16, BH, W], F16, tag="ftT")
                fold(st["Rbg"], ftB[0:16, :, :], nc.sync, nc.vector)
                fvB = pAB.tile([16, BH, W], F16, tag="fvT")
                nc.vector.scalar_tensor_tensor(
                    fvB[:], ftB[:], 0.0, rdiv0[:, n, :, :],
                    op0=mybir.AluOpType.bypass, op1=mybir.AluOpType.mult,
                    accum_out=pacc6[:, 2 * n:2 * n + 1])
                nc.sync.dma_start(d_fdiv[32 * n:32 * n + 16, :, :], fvB[:])

            st["c1datt"], st["conv2"], st["foldstt"] = c1datt, conv2, foldstt
            return st

        br = {n: make_branch(n) for n in (2, 1, 0)}
        br[2]["c1datt"]()
        br[2]["conv2"]()
        br[1]["c1datt"]()
        br[2]["foldstt"]()
        br[1]["conv2"]()
        br[0]["c1datt"]()
        br[1]["foldstt"]()
        br[0]["conv2"]()
        br[0]["foldstt"]()

        # ---------------- pool finalize ----------------
        tb = P.tile([16, 2], F32, tag="tb")
        pbg = P.tile([16, 1], F32, tag="pbg")
        ptg = P.tile([16, 1], F32, tag="ptg")
        nc.vector.tensor_add(tb[:, 0:1], pacc6[:, 0:1], pacc6[:, 2:3])
        nc.vector.tensor_add(pbg[:], tb[:, 0:1], pacc6[:, 4:5])
        nc.vector.tensor_add(tb[:, 1:2], pacc6[:, 1:2], pacc6[:, 3:4])
        nc.vector.tensor_add(ptg[:], tb[:, 1:2], pacc6[:, 5:6])
        nc.sync.dma_start(d_pool[0:16], pbg[:, 0])
        nc.sync.dma_start(d_pool[16:32], ptg[:, 0])

    nc.compile()
    return nc


def _build_B():
    nc = bacc.Bacc("TRN2", target_bir_lowering=False, debug=False,
                   num_devices=NCORES)

    d_fdiv = nc.dram_tensor("fdiv_in", [96, BH, W], F16, kind="ExternalInput")
    d_poolg = nc.dram_tensor("poolg", [80], F32, kind="ExternalInput")
    d_mw1 = nc.dram_tensor("mlpw1t", [80, 8], F32, kind="ExternalInput")
    d_mb1 = nc.dram_tensor("mlpb1", [8, 1], F32, kind="ExternalInput")
    d_mw2 = nc.dram_tensor("mlpw2t", [8, 96], F32, kind="ExternalInput")
    d_mb2 = nc.dram_tensor("mlpb2", [96, 1], F32, kind="ExternalInput")
    d_ones32 = nc.dram_tensor("ones32", [96, 32], F32, kind="ExternalInput")
    d_perm96 = nc.dram_tensor("perm96", [32, 96], F32, kind="ExternalInput")
    d_pat16 = nc.dram_tensor("pat16", [96, 32], F16, kind="ExternalInput")
    d_obg = nc.dram_tensor("out_bg", [C, BH, W], F16, kind="ExternalOutput")
    d_otg = nc.dram_tensor("out_tg", [C, BH, W], F16, kind="ExternalOutput")

    with tile.TileContext(nc) as tc, ExitStack() as ctx:
        P = ctx.enter_context(tc.tile_pool(name="persist", bufs=1))
        psB = ctx.enter_context(
            tc.tile_pool(name="psB", bufs=2, space=bass.MemorySpace.PSUM))
        psM = ctx.enter_context(
            tc.tile_pool(name="psM", bufs=2, space=bass.MemorySpace.PSUM))

        fdiv = P.tile([96, BH, W], F16, tag="fdiv")
        qs = [nc.sync, nc.gpsimd, nc.scalar, nc.sync]
        for q_i, q in enumerate(qs):
            r0 = 8 * q_i
            q.dma_start(fdiv[:, r0:r0 + 8, :], d_fdiv[:, r0:r0 + 8, :])
        poolg = P.tile([80, 1], F32, tag="poolg")
        nc.sync.dma_start(poolg[:, 0], d_poolg[:])
        mw1 = P.tile([80, 8], F32, tag="mw1")
        nc.scalar.dma_start(mw1[:], d_mw1[:])
        mb1 = P.tile([8, 1], F32, tag="mb1")
        nc.scalar.dma_start(mb1[:], d_mb1[:])
        mw2 = P.tile([8, 96], F32, tag="mw2")
        nc.scalar.dma_start(mw2[:], d_mw2[:])
        mb2 = P.tile([96, 1], F32, tag="mb2")
        nc.scalar.dma_start(mb2[:], d_mb2[:])
        ones32 = P.tile([96, 32], F32, tag="ones32")
        nc.gpsimd.dma_start(ones32[:], d_ones32[:])
        perm96 = P.tile([32, 96], F32, tag="perm96")
        nc.gpsimd.dma_start(perm96[:], d_perm96[:])
        pat16 = P.tile([96, 32], F16, tag="pat16")
        nc.gpsimd.dma_start(pat16[:], d_pat16[:])

        # merged two-stream MLP (block-diagonal weights), all N=1
        pm1 = psM.tile([96, 4], F32, tag="m")
        nc.tensor.matmul(pm1[0:8, 0:1], mw1[:, 0:8], poolg[:, 0:1],
                         start=True, stop=True)
        hdn = P.tile([8, 1], F32, tag="hdn")
        nc.scalar.activation(hdn[:], pm1[0:8, 0:1], RELU, bias=mb1[:])
        pm2 = psM.tile([96, 4], F32, tag="m")
        nc.tensor.matmul(pm2[0:96, 0:1], mw2[:, 0:96], hdn[:, 0:1],
                         start=True, stop=True)
        elog = P.tile([96, 1], F32, tag="elog")
        nc.scalar.activation(elog[:], pm2[0:96, 0:1], EXP, bias=mb2[:])
        pm3 = psM.tile([96, 4], F32, tag="m")
        nc.tensor.matmul(pm3[0:32, 0:1], ones32[:, 0:32], elog[:, 0:1],
                         start=True, stop=True)
        s3r = P.tile([32, 1], F32, tag="s3r")
        nc.vector.reciprocal(s3r[:], pm3[0:32, 0:1])
        pm4 = psM.tile([96, 4], F32, tag="m")
        nc.tensor.matmul(pm4[0:96, 0:1], perm96[:, 0:96], s3r[:, 0:1],
                         start=True, stop=True)
        wtl = P.tile([96, 1], F32, tag="wtl")
        nc.vector.tensor_mul(wtl[:], elog[:], pm4[0:96, 0:1])
        lhs96 = P.tile([96, 32], F16, tag="lhs96")
        nc.vector.tensor_scalar_mul(lhs96[:], pat16[:], wtl[:])

        # weighted combine on the PE: out[(si,c)] = sum_k wt[k]*fdiv[k]
        obuf = P.tile([32, BH, W], F16, tag="obuf")
        for q_i in range(4):
            r0 = 8 * q_i
            pq = psB.tile([32, 8, W], F32, tag="cmb")
            for h in (0, 4):
                nc.tensor.matmul(pq[0:32, h:h + 4, :], lhs96[:, 0:32],
                                 fdiv[:, r0 + h:r0 + h + 4, :],
                                 start=True, stop=True)
            nc.scalar.activation(obuf[:, r0:r0 + 8, :], pq[0:32, 0:8, :],
                                 COPY)
        nc.sync.dma_start(d_obg[:], obuf[0:16, :, :])
        nc.gpsimd.dma_start(d_otg[:], obuf[16:32, :, :])

    nc.compile()
    return nc


# ======================= host-side prep =======================

def _prep_core(inputs, b, k):
    y0 = BH * k
    m = {}
    for s in ("bg", "tg"):
        x = np.asarray(inputs[s])[b]            # [C, H, W]
        x8 = np.zeros((8, C, RX, CX), np.float16)
        for g in range(8):
            lo = y0 - 6 + g
            hi = lo + RX
            slo, shi = max(lo, 0), min(hi, H)
            if slo < shi:
                x8[g, :, slo - lo:shi - lo, 3:131] = x[:, slo:shi, :]
        m[f"x8{s}"] = x8.reshape(128, RX, CX)

    hug = np.zeros((3, RA, W), np.float32)
    for n, pad in enumerate(PADS):
        for r in range(RA):
            y = y0 - pad + r
            if not (0 <= y < H):
                hug[n, r, :] = 1e30
    m["hug"] = hug.astype(ml_dtypes.bfloat16)

    rdiv0 = np.zeros((16, 3, BH, W), np.float32)
    for n, pad in enumerate(PADS):
        yy = np.arange(H)
        rc = np.minimum(yy, pad) + np.minimum(H - 1 - yy, pad) + 1.0
        cc = np.minimum(yy[:W], pad) + np.minimum(W - 1 - yy[:W], pad) + 1.0
        div = np.outer(rc[y0:y0 + BH], cc) + 1e-8
        rdiv0[:, n] = (1.0 / div)[None, :, :]
    m["rdiv0"] = rdiv0.astype(np.float16)

    ones = np.zeros((73, 72), np.float32)
    ones[0:64, 0:64] = 1.0
    ones[64:72, 64:72] = 1.0
    ones[72, :] = 1.0
    m["oneslhs"] = ones.astype(ml_dtypes.bfloat16)

    for s, M, nmem in (("bg", NBG, "bg_mem"), ("tg", NTG, "tg_mem")):
        for n, p in enumerate(PS):
            mem = np.asarray(inputs[f"{nmem}{n}"])          # [M, C*p*p]
            temp = float(np.asarray(inputs[f"{s}_temp{n}"])[0])
            D = C * p * p
            arr = mem.reshape(M, C, p, p)
            w1 = arr.transpose(2, 1, 3, 0).reshape(p * C, p, M).copy()
            m[f"w1{s}{n}"] = (w1 * (temp / np.sqrt(D))).astype(np.float16)

    # fold consumes group q at row shift +q where q = 2*pad - i
    w2bg = np.zeros((2, NBG, 9, 8, 16), np.float32)
    base = [0, 2, 5]
    for n, p in enumerate(PS):
        pad = PADS[n]
        arr = np.asarray(inputs[f"bg_mem{n}"]).reshape(NBG, C, p, p)
        for ci in range((p + 1) // 2):
            for g in range(2):
                j = 2 * ci + g
                if j < p:
                    for i in range(p):
                        w2bg[g, :, base[n] + ci, 2 * pad - i, :] = \
                            arr[:, :, i, j]
    m["w2bg"] = w2bg.reshape(128, 9, 128).astype(np.float16)

    w2tg = np.zeros((8, NTG, 3, 8, 16), np.float32)
    for n, p in enumerate(PS):
        pad = PADS[n]
        arr = np.asarray(inputs[f"tg_mem{n}"]).reshape(NTG, C, p, p)
        for g in range(p):
            for i in range(p):
                w2tg[g, :, n, 2 * pad - i, :] = arr[:, :, i, g]
    m["w2tg"] = w2tg.reshape(64, 3, 128).astype(np.float16)
    return m


def _prep_B(inputs):
    """Per-batch-independent phase-B tensors (row layout: 32n+16si+c)."""
    m = {}
    mw1 = np.zeros((80, 8), np.float32)
    mb1 = np.zeros((8, 1), np.float32)
    mw2 = np.zeros((8, 96), np.float32)
    mb2 = np.zeros((96, 1), np.float32)
    for si, s in enumerate(("bg", "tg")):
        mw1[64 * si:64 * si + 16, 4 * si:4 * si + 4] = (
            np.asarray(inputs[f"{s}_fc1_w"]).T / (H * W))
        mb1[4 * si:4 * si + 4, 0] = np.asarray(inputs[f"{s}_fc1_b"])
        w2 = np.asarray(inputs[f"{s}_fc2_w"])          # [48, 4], row 16n+c
        b2 = np.asarray(inputs[f"{s}_fc2_b"])
        for n in range(3):
            for c in range(C):
                row = 32 * n + 16 * si + c
                mw2[4 * si:4 * si + 4, row] = w2[16 * n + c, :]
                mb2[row, 0] = b2[16 * n + c]
    ones32 = np.zeros((96, 32), np.float32)
    for n in range(3):
        for si in range(2):
            for c in range(C):
                ones32[32 * n + 16 * si + c, 16 * si + c] = 1.0
    m["mlpw1t"], m["mlpb1"] = mw1, mb1
    m["mlpw2t"], m["mlpb2"] = mw2, mb2
    m["ones32"] = ones32
    m["perm96"] = ones32.T.copy()
    m["pat16"] = ones32.astype(np.float16)
    return m


TRACE = False
TRACE_DIR = None


def kernel(**inputs):
    if "ncA" not in _CACHE:
        _CACHE["ncA"] = _build_A()
        _CACHE["ncB"] = _build_B()

    in_maps = []
    for core in range(NCORES):
        b, k = divmod(core, NBANDS)
        in_maps.append(_prep_core(inputs, b, k))

    kwA = {}
    kwB = {}
    if TRACE:
        kwA = dict(trace=True, tmpdir=(TRACE_DIR + "/A") if TRACE_DIR else None)
        kwB = dict(trace=True, tmpdir=(TRACE_DIR + "/B") if TRACE_DIR else None)

    resA = run_bass_kernel_spmd(_CACHE["ncA"], in_maps, list(range(NCORES)),
                                **kwA)

    # host glue: reduce pooled partials within each batch's 4-band group
    poolsum = {}
    for b in range(B):
        ps = np.sum([resA.results[b * NBANDS + k]["pool_out"]
                     for k in range(NBANDS)], axis=0).astype(np.float32)
        pg = np.zeros(80, np.float32)
        pg[0:16] = ps[0:16]
        pg[64:80] = ps[16:32]
        poolsum[b] = pg

    mB = _prep_B(inputs)
    in_mapsB = []
    for core in range(NCORES):
        b, k = divmod(core, NBANDS)
        d = dict(mB)
        d["fdiv_in"] = resA.results[core]["fdiv_out"]
        d["poolg"] = poolsum[b]
        in_mapsB.append(d)
    resB = run_bass_kernel_spmd(_CACHE["ncB"], in_mapsB, list(range(NCORES)),
                                **kwB)
    _CACHE["resA"] = resA
    _CACHE["resB"] = resB

    f_bc = np.zeros((B, C, H, W), np.float32)
    f_tg = np.zeros((B, C, H, W), np.float32)
    for core in range(NCORES):
        b, k = divmod(core, NBANDS)
        y0 = BH * k
        f_bc[b, :, y0:y0 + BH, :] = resB.results[core]["out_bg"].astype(np.float32)
        f_tg[b, :, y0:y0 + BH, :] = resB.results[core]["out_tg"].astype(np.float32)
    return (f_bc, f_tg)
